# revision 63
# baseline (speedup 1.0000x reference)
"""JointAttention TRN2 Bass kernel, v3.

Sharding: 8 cores = batch(2) x head-group(4); each core owns one batch
element and 4 heads (256 channels). Host sums the 4 row-parallel output
partials per batch element and adds bo.

Per-core dataflow (fp32 PSUM everywhere):
  - QK^T scores run as hybrid-precision fp8e4m3 DoubleRow matmuls
    (0.5 PE cycles/row): q and k are split on-device into fp8 hi + lo
    residual pairs (DVE tensor_scalar + scalar_tensor_tensor off the
    projection psum), and qh*kh + qh*kl + ql*kh is contracted in ONE
    [96 partitions x 2 slices] DoubleRow matmul per head per key chunk
    (fp16-level accuracy at half the PE cost of 64-deep fp16 scores).
    The (partition, slice) operand layout is assembled by small
    SBUF->SBUF shuffle DMAs on the Pool/SP queues.
  - exp runs per [128k, 512q] head-tile (one PSUM bank each, 4-deep
    rotation so each exp->scores WAR chain has a full iteration of
    slack). ~40% of exps run on DVE via a Schraudolph bit-trick
    (i16 = round(s*1024*log2e + 15*1024-60) bitcast to fp16, ~1.5% rms
    error) to unload the otherwise-saturated ACT engine; the rest are
    exact ACT exps. Error budget validated end-to-end: 7.4e-3 L2 vs the
    2e-2 gate.
  - PV in flipped [q, d] layout: lhsT = exp-scores chunk (stationary),
    rhs = V[128k, 65] giving out[q, 64 data + denominator]; 7+7+2 outs
    per PSUM bank, one accumulation group per bank.
  - attention loop over (qc of 512 q) x (kc of 128 keys) in three
    rounds (self keys 0-7, self 8-15, ctx) with PV partials spilled to
    SBUF between rounds (spill copies split ACT/DVE) so projection
    deadlines spread across the whole stream.
  - per-qc epilogue: psum+spill restore-add, reciprocal (DVE), softmax
    divide on Pool, DMA-transpose, output projection (stage copies
    alternate DVE/ACT).
  - k/v/q projections + fp8 operand builds for later chunks are JIT'd
    inside the attention loop (pop horizon it+6); scores are emitted
    three iterations ahead so exps park in engine wait queues.
"""

import sys
from contextlib import ExitStack

import numpy as np

if "/opt/trn_rl_repo" not in sys.path:
    sys.path.insert(0, "/opt/trn_rl_repo")

import concourse.bass as bass
import concourse.tile as tile
from concourse import bacc, mybir
from concourse.bass_utils import run_bass_kernel_spmd

F32 = mybir.dt.float32
F16 = mybir.dt.float16
I16 = mybir.dt.int16
FP8 = mybir.dt.float8e4
AFT = mybir.ActivationFunctionType
ALU = mybir.AluOpType
PM = mybir.MatmulPerfMode

# Schraudolph exp-approx constants (i16 = round(s*SCH_S + SCH_B) bitcast f16)
SCH_S = float(1024.0 / np.log(2.0))
SCH_B = float(15 * 1024 - 60)

D = 1024          # model dim
T = 2048          # query length (= self key length)
TK = 4096         # total key length (self + context)
CS = 256          # channels per core (4 heads x 64)
NH = 4            # heads per core
HD = 64           # head dim
DC = 8            # D chunks of 128 for the contraction
N_CORES = 8
QW = 512          # query chunk width (4 chunks)
NQC = T // QW
NKC = TK // 128   # 32 key chunks

# PV psum packing: out o = qb*4 + h -> bank o//7, lane o%7 (65 cols each)
PV_COL = {o: (o // 7) * 512 + (o % 7) * 65 for o in range(16)}
# compact SBUF ranges per qb: list of (psum_lo, psum_hi)
PV_QB_RANGES = {
    0: [(0, 260)],
    1: [(260, 455), (512, 577)],
    2: [(577, 837)],
    3: [(837, 967), (1024, 1154)],
}


def build_nc():
    nc = bacc.Bacc(None)

    xT = nc.declare_dram_parameter("xT", [D, T], F16, isOutput=False)
    cT = nc.declare_dram_parameter("cT", [D, T], F16, isOutput=False)
    wq = nc.declare_dram_parameter("wq", [D, CS], F16, isOutput=False)
    wks = nc.declare_dram_parameter("wks", [D, CS], F16, isOutput=False)
    wkc = nc.declare_dram_parameter("wkc", [D, CS], F16, isOutput=False)
    wvs = nc.declare_dram_parameter("wvs", [D, CS], F16, isOutput=False)
    wvc = nc.declare_dram_parameter("wvc", [D, CS], F16, isOutput=False)
    bq = nc.declare_dram_parameter("bq", [CS, 1], F32, isOutput=False)
    bks = nc.declare_dram_parameter("bks", [CS, 1], F32, isOutput=False)
    bkc = nc.declare_dram_parameter("bkc", [CS, 1], F32, isOutput=False)
    bvs = nc.declare_dram_parameter("bvs", [1, CS], F32, isOutput=False)
    bvc = nc.declare_dram_parameter("bvc", [1, CS], F32, isOutput=False)
    wo = nc.declare_dram_parameter("wo", [CS, D], F16, isOutput=False)
    out = nc.declare_dram_parameter("out", [T, D], F16, isOutput=True)

    with tile.TileContext(nc) as tc:
        _emit(nc, tc, xT, cT, wq, wks, wkc, wvs, wvc,
              bq, bks, bkc, bvs, bvc, wo, out)
    nc.compile()
    return nc


def _emit(nc, tc, xT, cT, wq, wks, wkc, wvs, wvc, bq, bks, bkc, bvs, bvc,
          wo, out):
    ctx = ExitStack()
    with ctx:
        consts = ctx.enter_context(tc.tile_pool(name="consts", bufs=1))
        io_pool = ctx.enter_context(tc.tile_pool(name="io", bufs=11))
        q8_pool = ctx.enter_context(tc.tile_pool(name="q8", bufs=1))
        k8_pool = ctx.enter_context(tc.tile_pool(name="k8", bufs=1))
        pre_pool = ctx.enter_context(tc.tile_pool(name="pre", bufs=8))
        v_pool = ctx.enter_context(tc.tile_pool(name="v", bufs=1))
        pt_pool = ctx.enter_context(tc.tile_pool(name="pt", bufs=16))
        pvs_pool = ctx.enter_context(tc.tile_pool(name="pvs", bufs=8))
        a_pool = ctx.enter_context(tc.tile_pool(name="a", bufs=8))
        at_pool = ctx.enter_context(tc.tile_pool(name="at", bufs=2))
        r_pool = ctx.enter_context(tc.tile_pool(name="r", bufs=8))
        stage_pool = ctx.enter_context(tc.tile_pool(name="stage", bufs=6))
        # PSUM: scores 2x[128,1024] (4 banks) + pv [128,1536] (3) + proj (1)
        ps_s = ctx.enter_context(
            tc.tile_pool(name="ps_s", bufs=4, space="PSUM"))
        ps_pv = ctx.enter_context(
            tc.tile_pool(name="ps_pv", bufs=1, space="PSUM"))
        ps_pj = ctx.enter_context(
            tc.tile_pool(name="ps_pj", bufs=1, space="PSUM"))

        # ---- constant DMAs (weights / biases), consumption order ----
        w_sb = {}
        b_sb = {}
        bv_sb = {}

        def load_w(name, w, eng=nc.sync):
            t = consts.tile([128, DC, CS], F16, tag=f"w_{name}",
                            name=f"w_{name}")
            eng.dma_start(out=t, in_=w.rearrange("(a p) c -> p a c", p=128))
            w_sb[name] = t

        def load_b(name, b, eng=nc.sync):
            t = consts.tile([128, 2], F32, tag=f"b_{name}", name=f"b_{name}")
            eng.dma_start(out=t, in_=b.rearrange("(a p) o -> p (a o)", p=128))
            b_sb[name] = t

        def load_bv(name, b, eng=nc.sync):
            t = consts.tile([128, CS], F32, tag=f"bv_{name}", name=f"bv_{name}")
            eng.dma_start(out=t, in_=b[:, :].to_broadcast([128, CS]))
            bv_sb[name] = t

        io_tiles = {}

        def load_window(src_i, src, w, split=False):
            # two DMAs per window (4 d-chunks each) — fewer dispatches,
            # bigger transfers.  split=True (prime window) uses four
            # half-size DMAs alternating queues so the first projection
            # matmuls start as soon as their own d-chunks land.
            ng = 4 if split else 2
            per = DC // ng
            for g in range(ng):
                if split:
                    t = consts.tile([128, per, 512], F16, tag=f"io0_{g}",
                                    name=f"io_{src_i}_{w}_{g}")
                else:
                    t = io_pool.tile([128, per, 512], F16, tag="io",
                                     name=f"io_{src_i}_{w}_{g}")
                eng = nc.scalar if (split and g % 2 == 1) else nc.sync
                eng.dma_start(
                    out=t,
                    in_=src[g * per * 128:(g + 1) * per * 128,
                            w * 512:(w + 1) * 512].rearrange(
                        "(a p) t -> p a t", p=128))
                for a in range(per):
                    io_tiles[(src_i, w, g * per + a)] = t[:, a, :]

        # first needs: wq + x window 0 (wq split across both queues so the
        # window transfers start earlier)
        t = consts.tile([128, DC, CS], F16, tag="w_wq", name="w_wq")
        nc.sync.dma_start(
            out=t[:, 0:4, :],
            in_=wq[0:512, :].rearrange("(a p) c -> p a c", p=128))
        nc.scalar.dma_start(
            out=t[:, 4:8, :],
            in_=wq[512:1024, :].rearrange("(a p) c -> p a c", p=128))
        w_sb["wq"] = t
        load_window(0, xT, 0, split=True)
        load_b("bq", bq)
        load_w("wks", wks, eng=nc.scalar)
        load_b("bks", bks, eng=nc.scalar)
        load_w("wvs", wvs, eng=nc.scalar)
        load_bv("bvs", bvs, eng=nc.scalar)

        def load_rest():
            # bulk preloads are emitted AFTER the prime projections so
            # the prime's q8/k8 shuffle DMAs aren't queued behind ~25us
            # of window transfers on the sync queue
            for w in range(1, 4):
                load_window(0, xT, w)
            load_w("wkc", wkc)
            load_b("bkc", bkc)
            load_w("wvc", wvc)
            load_bv("bvc", bvc)
            for w in range(4):
                load_window(1, cT, w)
            t = consts.tile([128, 2, D], F16, tag="wo", name="wo_sb")
            nc.sync.dma_start(out=t,
                              in_=wo.rearrange("(a p) f -> p a f", p=128))
            return t
        # identity for PE transposes, built on-device: (j - p) == 0
        ident_it = consts.tile([128, 128], mybir.dt.int32, tag="ident_it")
        nc.gpsimd.iota(ident_it, [[1, 128]], channel_multiplier=-1)
        ident_sb = consts.tile([128, 128], F16, tag="ident")
        nc.vector.tensor_scalar(ident_sb, ident_it, 0, None,
                                mybir.AluOpType.is_equal)

        # ---- persistent SBUF tensors ----
        # DoubleRow hybrid-fp8 score operands, per head h (96 partitions):
        #   p in [0,64):  slice0 = (qh[c], kh[c])   slice1 = (qh[c], kl[c])
        #   p in [64,96): slice0 = (ql[c0], kh[c0]) slice1 = (ql[c1], kh[c1])
        # with c = 64h+p, c0 = 64h+p-64, c1 = 64h+p-32, giving the full
        # hybrid contraction qh*kh + qh*kl + ql*kh over 64 channels in one
        # 0.5-cycle/row DoubleRow matmul.
        q8_sb = q8_pool.tile([96, 2, NH, T], FP8, tag="q8", name="q8")
        k8_sb = k8_pool.tile([96, 2, NH, TK], FP8, tag="k8", name="k8")
        v_sb = [v_pool.tile([128, NH * (HD + 1)], F16, tag=f"v{kc}",
                            name=f"v{kc}")
                for kc in range(NKC)]

        # proj psum slots: ping-pong with the pv banks while they are free
        pingpong = [True]  # mutable flag: True during prime

        def proj_ps(shape, name, dtype=F32):
            if pingpong[0]:
                proj_ps.rot = (proj_ps.rot + 1) % 4
                if proj_ps.rot:
                    return ps_pv.tile(shape, dtype,
                                      tag=f"pv{proj_ps.rot - 1}", name=name)
            return ps_pj.tile(shape, dtype, tag="pj", name=name)
        proj_ps.rot = -1

        # ---- projection emitters (bias adds on Pool to spare DVE).
        # Each unit is split into two 4-chunk halves so the JIT filler can
        # interleave at ~430ns granularity. ----
        # rotating DMA queue picker for the fp8 shuffle transfers
        shuf_eng = [0]

        hl_n = [0]

        def _hl_pre(ps, bias_ap, name):
            """hi/lo fp8 split of (ps + bias) into a pre tile, lane-aligned:
            pre[:,0,:] = fp8(ps+bias), pre[:,1,:] = fp8((ps+bias) - hi).
            The hi op alternates DVE/ACT to smooth DVE's per-unit bursts."""
            pre = pre_pool.tile([128, 2, 512], FP8, tag="pre", name=name)
            hl_n[0] += 1
            if hl_n[0] % 3 == 0:
                nc.scalar.activation(pre[:, 0, :], ps, AFT.Identity,
                                     bias=bias_ap)
            else:
                nc.vector.tensor_scalar_add(pre[:, 0, :], ps, bias_ap)
            nc.vector.scalar_tensor_tensor(
                pre[:, 1, :], ps, bias_ap, pre[:, 0, :],
                ALU.add, ALU.subtract)
            return pre

        def _k_half(box, src_i, w, cc, half, lo=0):
            wk, bk = ("wks", "bks") if src_i == 0 else ("wkc", "bkc")
            if half == 0:
                box["ps"] = proj_ps([128, 512 - lo], f"k_ps_{src_i}_{w}_{cc}")
            ps = box["ps"]
            for dc in range(half * 4, half * 4 + 4):
                nc.tensor.matmul(
                    ps, w_sb[wk][:, dc, cc * 128:(cc + 1) * 128],
                    io_tiles[(src_i, w, dc)][:, lo:512],
                    start=(dc == 0), stop=(dc == DC - 1))
            if half == 1:
                pre = _hl_pre(ps, b_sb[bk][:, cc:cc + 1],
                              f"kpre_{src_i}_{w}_{cc}")
                kw = slice(src_i * T + w * 512, src_i * T + w * 512 + 512)
                for hh in range(2):
                    h = cc * 2 + hh
                    sp = 64 * hh
                    eng = (nc.gpsimd, nc.sync)[shuf_eng[0] % 2]
                    shuf_eng[0] += 1
                    eng.dma_start(out=k8_sb[0:64, :, h, kw],
                                  in_=pre[sp:sp + 64, :, :])
                    eng.dma_start(out=k8_sb[64:96, 0, h, kw],
                                  in_=pre[sp:sp + 32, 0, :])
                    eng.dma_start(out=k8_sb[64:96, 1, h, kw],
                                  in_=pre[sp + 32:sp + 64, 0, :])

        def _q_half(box, w, cc, half):
            if half == 0:
                box["ps"] = proj_ps([128, 512], f"q_ps_{w}_{cc}")
            ps = box["ps"]
            for dc in range(half * 4, half * 4 + 4):
                nc.tensor.matmul(
                    ps, w_sb["wq"][:, dc, cc * 128:(cc + 1) * 128],
                    io_tiles[(0, w, dc)],
                    start=(dc == 0), stop=(dc == DC - 1))
            if half == 1:
                pre = _hl_pre(ps, b_sb["bq"][:, cc:cc + 1], f"qpre_{w}_{cc}")
                qw = slice(w * 512, (w + 1) * 512)
                for hh in range(2):
                    h = cc * 2 + hh
                    sp = 64 * hh
                    eng = (nc.gpsimd, nc.sync)[shuf_eng[0] % 2]
                    shuf_eng[0] += 1
                    eng.dma_start(out=q8_sb[0:64, 0, h, qw],
                                  in_=pre[sp:sp + 64, 0, :])
                    eng.dma_start(out=q8_sb[0:64, 1, h, qw],
                                  in_=pre[sp:sp + 64, 0, :])
                    eng.dma_start(out=q8_sb[64:96, 0, h, qw],
                                  in_=pre[sp:sp + 32, 1, :])
                    eng.dma_start(out=q8_sb[64:96, 1, h, qw],
                                  in_=pre[sp + 32:sp + 64, 1, :])

        def _v_half(box, src_i, w, sub, half):
            wv, bv = ("wvs", "bvs") if src_i == 0 else ("wvc", "bvc")
            kc = src_i * 16 + w * 4 + sub
            if half == 0:
                box["ps"] = proj_ps([128, CS], f"v_ps_{kc}")
            ps = box["ps"]
            for dc in range(half * 4, half * 4 + 4):
                nc.tensor.matmul(
                    ps, io_tiles[(src_i, w, dc)][:, sub * 128:(sub + 1) * 128],
                    w_sb[wv][:, dc, :],
                    start=(dc == 0), stop=(dc == DC - 1))
            if half == 1:
                vt = v_sb[kc]
                v3 = vt[:, 0:NH * (HD + 1)].rearrange("p (h x) -> p h x", h=NH)
                nc.vector.tensor_add(
                    v3[:, :, 0:HD],
                    ps[:, :].rearrange("p (h x) -> p h x", h=NH),
                    bv_sb[bv][:, :].rearrange("p (h x) -> p h x", h=NH))
                nc.gpsimd.memset(
                    v3[:, :, HD:HD + 1].rearrange("p h one -> p (h one)"), 1.0)

        def emit_q(w):
            for cc in range(2):
                box = {}
                _q_half(box, w, cc, 0)
                _q_half(box, w, cc, 1)

        def emit_k(src_i, w, cc):
            box = {}
            _k_half(box, src_i, w, cc, 0)
            _k_half(box, src_i, w, cc, 1)

        def emit_v(src_i, w, sub):
            box = {}
            _v_half(box, src_i, w, sub, 0)
            _v_half(box, src_i, w, sub, 1)

        def emit_outproj(qc, at_t, qb, tail=False):
            # psum->SBUF stage copies alternate DVE/ACT to balance load
            qt = qc * 4 + qb
            for fc in range(2):
                ps = proj_ps([128, 512], f"o_ps_{qt}_{fc}")
                for cc in range(2):
                    nc.tensor.matmul(
                        ps, at_t[:, cc, qb * 128:(qb + 1) * 128],
                        wo_sb[:, cc, fc * 512:(fc + 1) * 512],
                        start=(cc == 0), stop=(cc == 1))
                st = stage_pool.tile([128, 512], F16, tag="st",
                                     name=f"o_st_{qt}_{fc}")
                if (tail and (qb + fc) % 2 == 1) or (not tail and fc == 1):
                    nc.scalar.activation(st, ps, AFT.Copy)
                else:
                    nc.vector.tensor_copy(st, ps)
                if tail:
                    dma_eng = (nc.scalar, nc.gpsimd, nc.sync)[(qb * 2 + fc) % 3]
                else:
                    dma_eng = (nc.sync, nc.gpsimd)[fc]
                dma_eng.dma_start(
                    out=out[qt * 128:(qt + 1) * 128, fc * 512:(fc + 1) * 512],
                    in_=st)

        # ---- PE warm-up: start the p-state ramp clock immediately so the
        # prime projections run at full clock ----
        warm = consts.tile([128, 16], F16, tag="warm")
        nc.gpsimd.memset(warm[:, :], 0.0)
        wps = ps_pj.tile([128, 16], F32, tag="pj", name="warm_ps")
        nc.tensor.matmul(wps[0:16, :], warm[:, 0:16], warm,
                         start=True, stop=True)

        # ---- minimal prime: only the cc0 halves of qT(qc0) and k(self w0)
        # are needed for the FIRST scores matmul; cc1 is emitted between
        # sA and sB of iteration 0 (see emit_scores first=True) ----
        emit_q(0)
        for cc in range(2):
            emit_k(0, 0, cc)
        wo_sb = load_rest()
        pingpong[0] = False   # pv banks become live in the attention loop

        # ---- background (JIT) unit min-heap: (deadline_iter, seq, emit_fn) ----
        import heapq
        jit = []
        jit_seq = [0]

        def jit_push(deadline, fn):
            heapq.heappush(jit, (deadline, jit_seq[0], fn))
            jit_seq[0] += 1

        def jit_pop():
            heapq.heappop(jit)[2]()

        # Segment order: qc0..3 over self keys, then qc0..3 over ctx keys
        # (PV partials spill to SBUF between the two halves).  Self windows
        # are needed in the first 16 iterations; ctx windows only from
        # global iteration 64, so they spread across the earlier slack.
        def push_k(dl, src_i, w, cc):
            box = {}
            jit_push(dl, lambda: _k_half(box, src_i, w, cc, 0))
            jit_push(dl, lambda: _k_half(box, src_i, w, cc, 1))

        def push_v(dl, src_i, w, sub):
            box = {}
            jit_push(dl, lambda: _v_half(box, src_i, w, sub, 0))
            jit_push(dl, lambda: _v_half(box, src_i, w, sub, 1))

        def push_q(dl, w):
            for cc in range(2):
                box = {}
                jit_push(dl, lambda box=box, cc=cc: _q_half(box, w, cc, 0))
                jit_push(dl, lambda box=box, cc=cc: _q_half(box, w, cc, 1))

        for sub in range(4):
            push_v(sub - 1, 0, 0, sub)
        for w in range(1, 4):
            dl = 0 if w == 1 else 4 * w + 18
            push_k(dl, 0, w, 0)
            push_k(dl + 1, 0, w, 1)
            for sub in range(4):
                push_v(dl + 2 + sub, 0, w, sub)
        push_q(4, 1)
        push_q(12, 2)
        push_q(20, 3)
        # ctx spread over the slack before its first use at global iter 64
        j = 0
        for w in range(4):
            for cc in range(2):
                push_k(50 + j, 1, w, cc)
                j += 1
            for sub in range(4):
                push_v(50 + j, 1, w, sub)
                j += 1

        # first/last touch of each pv bank in PV emission order
        _emit_order = [qb * 4 + h for h in range(NH) for qb in range(4)]
        PV_FIRST = {o: False for o in range(16)}
        PV_LAST = {o: False for o in range(16)}
        _seen = set()
        for o in _emit_order:
            if o // 7 not in _seen:
                _seen.add(o // 7)
                PV_FIRST[o] = True
        _seen = set()
        for o in reversed(_emit_order):
            if o // 7 not in _seen:
                _seen.add(o // 7)
                PV_LAST[o] = True

        pend_pv = None          # (qc, kc, ptA, ptB)
        pv_tile = {}            # qc -> psum tile
        spill_tile = {}         # qc -> SBUF partial from the self half
        spill_pool = ctx.enter_context(tc.tile_pool(name="spill", bufs=5))

        # PV lives in three single-bank tiles so round-boundary spills
        # release each bank to the next segment as soon as its own spill
        # copy has read it (finer WAR than one 3-bank tile).
        PV_BANK_USED = (455, 455, 130)

        def emit_pv(qc, kc, pts):
            seg_start = kc in (0, 8, 16)
            seg_stop_kc = {7, 15, 31}
            if seg_start:
                pv_tile[qc] = [
                    ps_pv.tile([128, 512], F32, tag=f"pv{b}",
                               name=f"pv_{qc}_{kc}_{b}")
                    for b in range(3)]
            pv = pv_tile[qc]
            for h in range(NH):
                for qb in range(4):
                    o = qb * 4 + h
                    col = PV_COL[o]
                    nc.tensor.matmul(
                        pv[col // 512][:, col % 512:col % 512 + 65],
                        pts[h][:, qb * 128:(qb + 1) * 128],
                        v_sb[kc][:, h * 65:(h + 1) * 65],
                        start=(seg_start and PV_FIRST[o]),
                        stop=(kc in seg_stop_kc and PV_LAST[o]))

        def emit_spill(qc, accum=False):
            pv = pv_tile.pop(qc)
            sp = spill_pool.tile([128, 1040], F32, tag="sp",
                                 name=f"spill_{qc}_{int(accum)}")
            old = spill_tile.get(qc)
            offs = (0, 455, 910)
            for b in (0, 1, 2):
                n, off = PV_BANK_USED[b], offs[b]
                if accum:
                    nc.vector.tensor_add(sp[:, off:off + n],
                                         pv[b][:, 0:n],
                                         old[:, off:off + n])
                elif b == 0:
                    # plain-copy spills split DVE/ACT so the pv banks
                    # release fast at segment boundaries
                    nc.scalar.activation(sp[:, off:off + n], pv[b][:, 0:n],
                                         AFT.Copy)
                else:
                    nc.vector.tensor_copy(sp[:, off:off + n],
                                          pv[b][:, 0:n])
            spill_tile[qc] = sp

        def emit_epilogue(qc, tail=False):
            """Per-qb pipelined: copy -> recip -> muls -> transpose; the
            out-projection for each qb is queued as a JIT unit.  The tail
            (last qc) uses PE transposes + Pool copies to dodge the DMA
            init latency, and emits everything inline."""
            pv = pv_tile.pop(qc)
            sp = spill_tile.pop(qc)
            at_t = at_pool.tile([128, 2, 512], F16, tag="at", name=f"at_{qc}")
            for qb in range(4):
                # pvs = ctx-half psum + self-half spill (restore-add)
                pvs = pvs_pool.tile([128, 260], F16, tag="pvs",
                                    name=f"pvs_{qc}_{qb}")
                off = 0
                for lo, hi in PV_QB_RANGES[qb]:
                    nc.vector.tensor_add(
                        pvs[:, off:off + hi - lo],
                        pv[lo // 512][:, lo % 512:lo % 512 + hi - lo],
                        sp[:, qb * 260 + off:qb * 260 + off + hi - lo])
                    off += hi - lo
                r = r_pool.tile([128, 4], F32, tag="r",
                                name=f"r_{qc}_{qb}")
                den_ap = pvs[:, 0:260].rearrange(
                    "p (c x) -> p c x", c=4)[:, :, HD:HD + 1].rearrange(
                    "p c one -> p (c one)")
                nc.vector.reciprocal(r, den_ap)
                a_t = a_pool.tile([128, CS], F16, tag="a",
                                  name=f"a_{qc}_{qb}")
                for h in range(NH):
                    eng = nc.gpsimd if (tail and h >= 2) or not tail \
                        else nc.vector
                    eng.tensor_scalar_mul(
                        a_t[:, h * HD:(h + 1) * HD],
                        pvs[:, h * 65:h * 65 + HD],
                        r[:, h:h + 1])
                teng = nc.scalar if tail else nc.sync
                for cc in range(2):
                    teng.dma_start_transpose(
                        at_t[:, cc, qb * 128:(qb + 1) * 128],
                        a_t[:, cc * 128:(cc + 1) * 128])
                if tail:
                    emit_outproj(qc, at_t, qb, tail=True)
                else:
                    jit_push(84 + 11 * qc + 2 * qb,
                             lambda qc=qc, at_t=at_t, qb=qb:
                             emit_outproj(qc, at_t, qb))

        # ---- attention loop: self halves for all qc, then ctx halves.
        # Scores/exps are emitted one iteration AHEAD of PV so they park in
        # the PE wait queue and issue the moment the previous exp frees
        # their psum slot; PV and JIT work fills the wait. ----
        iters = ([(qc, kc) for qc in range(NQC) for kc in range(8)]
                 + [(qc, kc) for qc in range(NQC) for kc in range(8, 16)]
                 + [(qc, kc) for qc in range(NQC)
                    for kc in range(16, 32)])

        def emit_exp(pt, s, on_dve):
            """exp(s) -> pt (f16).  ACT: exact exp.  DVE: Schraudolph
            bit-trick (i16 = round(s*S+B), bitcast f16, ~1.5% rms err)."""
            if on_dve:
                nc.vector.tensor_scalar(pt[:, :].bitcast(I16), s,
                                        SCH_S, SCH_B, ALU.mult, ALU.add)
            else:
                nc.scalar.activation(pt, s, AFT.Exp)

        def emit_scores(qc, kc, it=-1, first=False):
            # One [128,512] psum tile + one exp per head: 4 independent
            # exp->mm WAR chains (one per psum bank), each with a full
            # iteration of slack, so the chain latency stays under the PE
            # period.  DVE takes a spread subset of the exps to unload ACT;
            # spill/epilogue iters stay on ACT where DVE is busy.
            qs = slice(qc * QW, (qc + 1) * QW)
            ks = slice(kc * 128, (kc + 1) * 128)
            # per-iter exp split, phase-dependent: DVE's hl-pre/spill load
            # concentrates in the self rounds (it < 64), so it takes fewer
            # exps there and more during the ctx round where it is idle.
            if kc in (7, 15, 31):
                dve_h = {it % NH, (it + 2) % NH}
            else:
                dve_h = {h for h in range(NH) if ((it * 4 + h) * 2) % 5 < 2}
            pts = []
            for h in range(NH):
                s_h = ps_s.tile([128, 512], F32, tag="s",
                                name=f"s_{qc}_{kc}_{h}")
                nc.tensor.matmul(s_h, k8_sb[0:96, :, h, ks],
                                 q8_sb[0:96, :, h, qs],
                                 start=True, stop=True,
                                 perf_mode=PM.DoubleRow)
                pt_h = pt_pool.tile([128, 512], F16, tag="pt",
                                    name=f"pt_{qc}_{kc}_{h}")
                emit_exp(pt_h, s_h, h in dve_h)
                pts.append(pt_h)
            return pts

        # scores are emitted TWO iterations ahead: the psum WAR chain
        # exp(it) -> scores-mm(it+2) then spans two iterations, so the
        # exp+sem latency is amortized and PE stays the pacing engine.
        pend = [emit_scores(*iters[0], it=0, first=True),
                emit_scores(*iters[1], it=1),
                emit_scores(*iters[2], it=2)]
        for it in range(len(iters)):
            qc, kc = iters[it]
            if it + 3 < len(iters):
                pend.append(emit_scores(*iters[it + 3], it=it + 3))
            while jit and jit[0][0] <= it + 6:
                jit_pop()
            emit_pv(qc, kc, pend.pop(0))
            if kc == 7:
                emit_spill(qc)
            elif kc == 15:
                emit_spill(qc, accum=True)
            elif kc == NKC - 1 and qc != NQC - 1:
                emit_epilogue(qc)

        while jit:
            jit_pop()
        pingpong[0] = True     # pv banks free again for the tail
        emit_epilogue(NQC - 1, tail=True)


_NC_CACHE = None


def _in_maps(inputs):
    f32 = {k: np.asarray(v, dtype=np.float32) for k, v in inputs.items()}
    x, context = f32["x"], f32["context"]
    B = x.shape[0]
    xTs = [np.ascontiguousarray(x[b].T).astype(np.float16) for b in range(B)]
    cTs = [np.ascontiguousarray(context[b].T).astype(np.float16)
           for b in range(B)]
    in_maps = []
    for b in range(B):
        for hg in range(4):
            sl = slice(hg * CS, (hg + 1) * CS)
            in_maps.append({
                "xT": xTs[b],
                "cT": cTs[b],
                "wq": (np.ascontiguousarray(f32["Wq"][:, sl]) * 0.125
                       ).astype(np.float16),
                "wks": np.ascontiguousarray(f32["Wks"][:, sl]
                                            ).astype(np.float16),
                "wkc": np.ascontiguousarray(f32["Wkc"][:, sl]
                                            ).astype(np.float16),
                "wvs": np.ascontiguousarray(f32["Wvs"][:, sl]
                                            ).astype(np.float16),
                "wvc": np.ascontiguousarray(f32["Wvc"][:, sl]
                                            ).astype(np.float16),
                "bq": (f32["bq"][sl] * 0.125).reshape(CS, 1).copy(),
                "bks": f32["bks"][sl].reshape(CS, 1).copy(),
                "bkc": f32["bkc"][sl].reshape(CS, 1).copy(),
                "bvs": f32["bvs"][sl].reshape(1, CS).copy(),
                "bvc": f32["bvc"][sl].reshape(1, CS).copy(),
                "wo": np.ascontiguousarray(f32["Wo"][sl, :]
                                           ).astype(np.float16),
            })
    return in_maps


def kernel(**inputs):
    global _NC_CACHE
    if _NC_CACHE is None:
        _NC_CACHE = build_nc()
    nc = _NC_CACHE

    in_maps = _in_maps(inputs)
    res = run_bass_kernel_spmd(nc, in_maps, list(range(N_CORES))).results

    bo = np.asarray(inputs["bo"], dtype=np.float32)
    B = np.asarray(inputs["x"]).shape[0]
    out = np.empty((B, T, D), dtype=np.float32)
    for b in range(B):
        acc = res[b * 4 + 0]["out"].astype(np.float32).copy()
        for hg in range(1, 4):
            acc += res[b * 4 + hg]["out"]
        out[b] = acc + bo
    return out

